# revision 2
# baseline (speedup 1.0000x reference)
"""LocallyConnected2d v4 (3x3, 64x64 out, C_in=16, C_out=32, B=32) on 8 trn2 cores.

out[b,o,h,w] = sum_{c,i,j} x[b,c,h+i,w+j] * weight[0,o,c,h,w,(i,j)] + bias[0,o,h,w]

Sharding: spatial over H_out - core i computes output rows 8i..8i+8.

Design:
- pa holds 8 shifted tap bands (taps 0..7, 16 channels each) on 128
  partitions; band k at flat offset +1 equals tap k+1 within a kernel row,
  so tap 8 is band 7 read at +1 - no extra x tensor.
- Quad packing: 4 positions/matmul, lhsT [128, 4x32], rhs [128, 4x32]; the
  diagonal 32x32 blocks of the [128,128] product are the outputs.
- Contraction K=144 as TWO K=128 matmuls PSUM-accumulating per quad slot:
  A = taps 0..7 at offset 0; B = offset +1 with rows 0..111 zero-weighted
  and rows 112..127 = tap-8 weights. K=128 keeps the fast-weight-load path
  (measured 2x on LDWEIGHTS+MM pairs vs K=112/K=32 splits). The zero rows
  live in persistent SBUF tiles memset once; only 16 real rows stream from
  HBM per pass.
- Bias is fused into the single DVE PSUM->SBUF copy (tensor_tensor add with
  stride-0 broadcast AP).
- Stage layout [128, 4(p), 32(s), 4(q), 32(b)] bf16 makes each diagonal
  out-slice contiguous per partition; 8 compact out-DMAs (jw x s-half).
- Benchmark loop (n_iters>1) unrolls TWO logical iterations per hardware
  For_i pass with alternating tile sets, so one set's input DMAs overlap the
  other set's compute (tile addresses inside a hardware loop are static, so
  pool double-buffering alone cannot overlap across iterations).
"""

import contextlib

import numpy as np

import concourse.bass as bass
import concourse.mybir as mybir
import concourse.tile as tile
from concourse import bacc
from concourse import bass_utils

N_CORES = 8
B, CI, CO = 32, 16, 32
H = W = 64
HL = H // N_CORES          # output rows per core
XROWS = HL + 2             # input rows per core (with halo)
XW = 66
XFLAT = XROWS * XW         # 660
T = HL * XW                # 528 flat window positions (8 chunks of 66)
KA = 128                   # taps 0..7
KB = 16                    # tap 8 (rows 112..127 of the K=128 B-matmul)

_cache = {}


def _np_dt():
    import ml_dtypes
    return np.dtype(ml_dtypes.bfloat16)


def _build(n_iters=1, mode="full"):
    assert mode in ("full", "dma", "penodve")
    do_pe = mode != "dma"
    do_dve = mode == "full"
    dt = mybir.dt.bfloat16
    f32 = mybir.dt.float32
    nc = bacc.Bacc("TRN2", target_bir_lowering=False, debug=False,
                   num_devices=N_CORES)
    xr_d = nc.dram_tensor("xr", [128, T, B], dt, kind="ExternalInput")
    wa_d = nc.dram_tensor("wa", [HL, KA, W, CO], dt, kind="ExternalInput")
    wb_d = nc.dram_tensor("wb", [HL, KB, W, CO], dt, kind="ExternalInput")
    bias_d = nc.dram_tensor("biasq", [128, HL, 4, 4], dt, kind="ExternalInput")
    # out chunk (jw, sh): stage[32jw:32jw+32, jw, 16sh:16sh+16, :, :]
    # s = 4*h + gq; w = 16*gq + 4*q + jw
    out_d = nc.dram_tensor("out", [4, 32, 32, 4, 32], dt,
                           kind="ExternalOutput")

    nsets = 1 if n_iters == 1 else 2
    if n_iters > 1:
        assert n_iters % 2 == 0, "benchmark loop needs even n_iters"

    with tile.TileContext(nc) as tc:
        with (
            tc.tile_pool(name="px", bufs=1) as px,
            tc.tile_pool(name="pw", bufs=1) as pw,
            tc.tile_pool(name="pwb", bufs=1) as pwb,
            tc.tile_pool(name="pbias", bufs=1) as pbias,
            tc.tile_pool(name="pst", bufs=1) as pst,
            tc.tile_pool(name="pp", bufs=8, space=bass.MemorySpace.PSUM) as pp,
        ):
            # Persistent B-weight tiles: rows 0..111 zeroed once outside the
            # loop; each pass only DMAs the 16 real tap-8 rows.
            wbs, biases = [], []
            for s in range(nsets):
                wb_s = pwb.tile([128, HL, W, CO], dt, tag=f"wb{s}",
                                name=f"wb{s}")
                nc.gpsimd.memset(wb_s[0:112, :, :, :], 0.0)
                wbs.append(wb_s)
                bias_s = pbias.tile([128, HL, 4, 4], dt, tag=f"bias{s}",
                                    name=f"bias{s}")
                biases.append(bias_s)
            stage = pst.tile([128, 4, 32, 4, 32], dt, tag="stage",
                             name="stage")
            if not do_pe:
                nc.vector.memset(stage[:], 0.0)

            pas = []
            for s in range(nsets):
                pa_s = px.tile([128, T, B], dt, tag=f"pa{s}", name=f"pa{s}")
                pas.append(pa_s)

            def emit_inputs(s):
                pa = pas[s]
                half = T // 2
                nc.sync.dma_start(pa[:, 0:half, :], xr_d[:, 0:half, :])
                nc.scalar.dma_start(pa[:, half:T, :], xr_d[:, half:T, :])

            def emit_compute(s):
                pa, wb_s, bias_s = pas[s], wbs[s], biases[s]
                was = []
                for c in range(4):
                    wa_c = pw.tile([KA, 2, W, CO], dt, tag=f"wa{c}",
                                   name=f"wa{c}")
                    eng = nc.sync if c % 2 == 0 else nc.scalar
                    eng.dma_start(wa_c[:], wa_d[2 * c:2 * c + 2].rearrange(
                        "h k w o -> k h w o"))
                    was.append(wa_c)
                for h in range(HL):
                    wa_h = was[h // 2][:, h % 2]
                    if not do_pe:
                        continue
                    for gp in range(2):
                        # two PSUM banks per tile; one DVE add covers both
                        bank = pp.tile([128, 8, 4, 32], f32, tag="bank",
                                       name="bank", bufs=4)
                        for q8 in range(8):
                            gq = 2 * gp + q8 // 4
                            q = q8 % 4
                            w0 = 16 * gq + 4 * q
                            t0 = XW * h + w0
                            nc.tensor.matmul(
                                bank[:, q8, :, :],
                                wa_h[:, w0:w0 + 4, :],
                                pa[:, t0:t0 + 4, :],
                                start=True, stop=False)
                            nc.tensor.matmul(
                                bank[:, q8, :, :],
                                wb_s[:, h, w0:w0 + 4, :],
                                pa[:, t0 + 1:t0 + 5, :],
                                start=False, stop=True)
                        sl = 4 * h + 2 * gp
                        if do_dve:
                            bias_bc = (bias_s[:, h, 2 * gp:2 * gp + 2, :]
                                       .unsqueeze(1)
                                       .rearrange("k p s q -> k p (s q)")
                                       .unsqueeze(-1)
                                       .broadcast_to((128, 4, 8, 32)))
                            nc.vector.tensor_tensor(
                                stage[:, :, sl:sl + 2, :, :]
                                .rearrange("k p s q b -> k p (s q) b"),
                                bank[:].rearrange("k sq p b -> k p sq b"),
                                bias_bc,
                                op=mybir.AluOpType.add)
                        elif sl == 0:
                            nc.vector.tensor_copy(
                                stage[:, :, sl:sl + 2, :, :]
                                .rearrange("k p s q b -> k p (s q) b"),
                                bank[:].rearrange("k sq p b -> k p sq b"))
                    if h in (3, 7):
                        emit_outputs(h // 4)

            def emit_outputs(sh):
                for jw in range(4):
                    oeng = nc.sync if (jw + sh) % 2 == 0 else nc.scalar
                    oeng.dma_start(
                        out_d[jw, :, 16 * sh:16 * sh + 16, :, :].opt(),
                        stage[32 * jw:32 * jw + 32, jw,
                              16 * sh:16 * sh + 16, :, :].opt())

            def emit_wb(s):
                nc.gpsimd.dma_start(wbs[s][112:128, :, :, :],
                                    wb_d[:].rearrange("h k w o -> k h w o"))
                nc.gpsimd.dma_start(biases[s][:], bias_d[:])

            if n_iters == 1:
                emit_inputs(0)
                emit_wb(0)
                emit_compute(0)
            else:
                # pa/wb/bias are loaded one pass ahead (the loop reloads the
                # same values every pass), so compute never waits on input
                # DMAs at a pass boundary.
                emit_inputs(0)
                emit_inputs(1)
                emit_wb(0)
                emit_wb(1)
                with tc.For_i(0, n_iters // 2, 1):
                    emit_compute(0)
                    emit_inputs(0)
                    emit_wb(0)
                    emit_compute(1)
                    emit_inputs(1)
                    emit_wb(1)
    nc.compile()
    return nc


def _get_nc(n_iters=1, mode="full"):
    key = (n_iters, mode)
    if key not in _cache:
        _cache[key] = _build(n_iters, mode)
    return _cache[key]


def _pack_inputs(x, weight, bias, use_bf16=True):
    """Full inputs -> per-core in_maps (host-side shard + relayout)."""
    np_dt = _np_dt()
    x = np.asarray(x, np.float32)
    weight = np.asarray(weight, np.float32)
    bias = np.asarray(bias, np.float32)

    # weight[0]: [o, c, h, w, k] -> [h, w, k, c, o]
    wperm = weight[0].transpose(2, 3, 4, 1, 0)
    bt = bias[0]                                       # [o, h, w]

    in_maps = []
    for core in range(N_CORES):
        r0 = HL * core
        xs = x[:, :, r0:r0 + XROWS, :].transpose(1, 0, 2, 3).reshape(
            CI, B, XFLAT)                              # [c, b, flat]
        pa = np.zeros((128, T, B), np.float32)
        for k in range(8):
            i, j = divmod(k, 3)
            off = XW * i + j
            blk = xs[:, :, off:off + T - 1]            # [16, 32, 527]
            pa[16 * k:16 * (k + 1), :T - 1, :] = blk.transpose(0, 2, 1)

        wc = wperm[r0:r0 + HL]                         # [h, w, 9, c, o]
        wa = wc[:, :, 0:8].reshape(HL, W, KA, CO).transpose(0, 2, 1, 3)
        wb = wc[:, :, 8:9].reshape(HL, W, KB, CO).transpose(0, 2, 1, 3)

        # bias_q[p=32*jw+o, h, gq, q] = bias[o, r0+h, 16*gq+4*q+jw]
        jw = (np.arange(128) // 32)                    # [128]
        o = (np.arange(128) % 32)
        gq = np.arange(4)
        q = np.arange(4)
        wcol = (16 * gq[None, :, None] + 4 * q[None, None, :]
                + jw[:, None, None])                   # [128, 4, 4]
        biasq = bt[o[:, None, None, None],
                   (r0 + np.arange(HL))[None, :, None, None],
                   wcol[:, None, :, :]]                # [128, 8, 4, 4]

        in_maps.append({
            "xr": np.ascontiguousarray(pa, dtype=np_dt),
            "wa": np.ascontiguousarray(wa, dtype=np_dt),
            "wb": np.ascontiguousarray(wb, dtype=np_dt),
            "biasq": np.ascontiguousarray(biasq, dtype=np_dt),
        })
    return in_maps


def _gather(results):
    # per-core out: [4(jw), 32(o), 32(s), 4(q), 32(b)], s = 4*h + gq,
    # w = 16*gq + 4*q + jw
    out = np.zeros((B, CO, H, W), np.float32)
    for core in range(N_CORES):
        oc = np.asarray(results[core]["out"], dtype=np.float32)
        oc = oc.reshape(4, CO, HL, 4, 4, B)      # [jw, o, h, gq, q, b]
        oc = oc.transpose(5, 1, 2, 3, 4, 0)      # [b, o, h, gq, q, jw]
        out[:, :, HL * core:HL * (core + 1), :] = oc.reshape(B, CO, HL, W)
    return out


def run(x, weight, bias, use_bf16=True, n_iters=1, mode="full", **spmd_kwargs):
    nc = _get_nc(n_iters, mode)
    in_maps = _pack_inputs(x, weight, bias)
    res = bass_utils.run_bass_kernel_spmd(nc, in_maps,
                                          core_ids=list(range(N_CORES)),
                                          **spmd_kwargs)
    return _gather(res.results), res


def kernel(x, weight, bias):
    out, _ = run(x, weight, bias)
    return out
